# revision 10
# baseline (speedup 1.0000x reference)
"""Baseline (v1) kernel, reconstructed for device sanity checks."""

import sys

for _p in ("/opt/trn_rl_repo",):
    if _p not in sys.path:
        sys.path.insert(0, _p)

import numpy as np
import ml_dtypes

import concourse.bass as bass
import concourse.tile as tile
from concourse import bacc, mybir
from concourse.bass_utils import run_bass_kernel_spmd

B, T_DEC, T_ENC, H2 = 8, 64, 512, 512
P = 128
NB = H2 // P

BF16 = mybir.dt.bfloat16
F32 = mybir.dt.float32
AF = mybir.ActivationFunctionType
ALU = mybir.AluOpType

_CACHE = {}

from contextlib import ExitStack

BA_W = H2 + T_DEC


def build_raw(bacc, mybir, bass):
    BF16 = mybir.dt.bfloat16
    F32 = mybir.dt.float32
    AF = mybir.ActivationFunctionType
    ALU = mybir.AluOpType

    nc = bacc.Bacc(None, target_bir_lowering=False)

    encN = nc.dram_tensor("encN", [T_ENC, H2], BF16, kind="ExternalInput")
    a1blk = nc.dram_tensor("a1blk", [NB, 2, P, H2], BF16, kind="ExternalInput")
    blkA = nc.dram_tensor("blkA", [NB, P, BA_W], BF16, kind="ExternalInput")
    b4d = nc.dram_tensor("b4", [P, NB], F32, kind="ExternalInput")
    out = nc.dram_tensor("out", [H2, T_DEC], F32, kind="ExternalOutput")

    encN_r = encN[:, :].rearrange("(a p) o -> p a o", p=P)
    a1_r = a1blk[:, :, :, :].rearrange("a two p c -> p a two c")
    bA_r = blkA[:, :, :].rearrange("a p c -> p a c")
    out_r = out[:, :].rearrange("(a p) i -> p a i", p=P)

    with ExitStack() as ctx:
        ec = ctx.enter_context
        a1t = [ec(nc.sbuf_tensor(f"a1t{d}", [P, 2, H2], BF16)) for d in range(NB)]
        bt = ec(nc.sbuf_tensor("bt", [P, NB, BA_W], BF16))
        eN = ec(nc.sbuf_tensor("eN", [P, NB, H2], BF16))
        b4 = ec(nc.sbuf_tensor("b4s", [P, NB], F32))
        ee = [ec(nc.sbuf_tensor(f"ee{o}", [P, T_ENC], BF16)) for o in range(NB)]
        ed = [ec(nc.sbuf_tensor(f"ed{o}", [P, T_DEC], BF16)) for o in range(NB)]
        lt = [ec(nc.sbuf_tensor(f"lt{j}", [P, T_DEC + 1], BF16)) for j in range(NB)]
        junk = ec(nc.sbuf_tensor("junk", [P, NB, T_ENC], BF16))
        jbf = ec(nc.sbuf_tensor("jbf", [P, T_ENC], BF16))
        wj = ec(nc.sbuf_tensor("wj", [P, NB], F32))
        cp = ec(nc.sbuf_tensor("cp", [P, NB], F32))
        se = ec(nc.sbuf_tensor("se", [P, NB], F32))
        fx = ec(nc.sbuf_tensor("fx", [P, NB], F32))
        ctmp = ec(nc.sbuf_tensor("ctmp", [P, NB, T_DEC], F32))
        ctxo = ec(nc.sbuf_tensor("ctxo", [P, NB, T_DEC], F32))
        pp = [ec(nc.psum_tensor(f"pp{o}", [P, T_ENC], F32)) for o in range(NB)]
        pd = ec(nc.psum_tensor("pd", [P, NB, T_DEC], F32))
        ps = ec(nc.psum_tensor("ps", [P, NB, T_DEC], F32))
        pcA = ec(nc.psum_tensor("pcA", [P, 2, T_DEC + 1], F32))
        pcB = ec(nc.psum_tensor("pcB", [P, 2, T_DEC + 1], F32))

        def pc(ob):
            return (pcA if ob < 2 else pcB)[:, ob % 2, :]

        def wte(db):
            return a1t[db][:, 0, :]

        def eT(db):
            return a1t[db][:, 1, :]

        jz = ec(nc.semaphore("jz"))
        dS = [ec(nc.semaphore(f"dS{d}")) for d in range(NB)]
        gA1 = ec(nc.semaphore("gA1"))
        gA2 = ec(nc.semaphore("gA2"))
        gA3 = ec(nc.semaphore("gA3"))
        dO = ec(nc.semaphore("dO"))
        pe = ec(nc.semaphore("pe"))
        ac = ec(nc.semaphore("ac"))
        dv = ec(nc.semaphore("dv"))

        with nc.Block(no_gpsimd_drain=True) as block:

            @block.sync
            def _(sync):
                for db in range(NB):
                    sync.dma_start(
                        out=a1t[db][:, :, :], in_=a1_r[:, db, :, :]
                    ).then_inc(dS[db], 16)
                sync.wait_ge(dv, 20)
                sync.dma_start(out=out_r[:, 0:2, :], in_=ctxo[:, 0:2, :]).then_inc(
                    dO, 16
                )
                sync.wait_ge(dv, 28)
                sync.dma_start(out=out_r[:, 2:NB, :], in_=ctxo[:, 2:NB, :]).then_inc(
                    dO, 16
                )
                sync.wait_ge(dO, 32)

            @block.gpsimd
            def _(gpsimd):
                gpsimd.dma_start(out=eN[:, :, :], in_=encN_r[:, :, :]).then_inc(
                    gA3, 16
                )

            @block.scalar
            def _(scalar):
                scalar.dma_start(out=bt[:, :, :], in_=bA_r[:, :, :]).then_inc(gA1, 16)
                scalar.dma_start(out=b4[:, :], in_=b4d[:, :]).then_inc(gA2, 16)
                scalar.activation(wj[:, 0:1], wj[:, 3:4], AF.Exp, scale=0.0).then_inc(
                    ac, 1
                )
                scalar.wait_ge(pe, NB)
                for ob in range(NB):
                    scalar.activation(ed[ob][:, :], pd[:, ob, :], AF.Exp).then_inc(
                        ac, 1
                    )
                scalar.wait_ge(gA2, 16)
                for ob in range(NB):
                    scalar.wait_ge(pe, NB + 1 + ob)
                    scalar.activation(
                        ee[ob][:, :],
                        pp[ob][:, :],
                        AF.Exp,
                        bias=b4[:, ob : ob + 1],
                    ).then_inc(ac, 1)
                scalar.activation(wj[:, 1:2], wj[:, 3:4], AF.Ln, bias=1.0, scale=0.0)
                scalar.wait_ge(pe, 3 * NB)
                for jb in range(NB):
                    scalar.activation(lt[jb][:, 0:T_DEC], ps[:, jb, :], AF.Ln).then_inc(
                        ac, 1
                    )

            @block.tensor
            def _(tensor):
                tensor.wait_ge(jz, 1)
                for k in range(10):
                    tensor.matmul(
                        pp[k % NB][:, :],
                        lhsT=jbf[:, 0:P],
                        rhs=jbf[:, :],
                        start=True,
                        stop=True,
                    )
                tensor.wait_ge(gA1, 16)
                for ob in range(NB):
                    for db in range(NB):
                        mm = tensor.matmul(
                            pd[:, ob, :],
                            lhsT=bt[:, db, ob * P : (ob + 1) * P],
                            rhs=bt[:, db, H2 : H2 + T_DEC],
                            start=(db == 0),
                            stop=(db == NB - 1),
                        )
                        if db == NB - 1:
                            mm.then_inc(pe, 1)
                for db in range(NB):
                    tensor.wait_ge(dS[db], 16)
                    for ob in range(NB):
                        mm = tensor.matmul(
                            pp[ob][:, :],
                            lhsT=wte(db)[:, ob * P : (ob + 1) * P],
                            rhs=eT(db)[:, :],
                            start=(db == 0),
                            stop=(db == NB - 1),
                        )
                        if db == NB - 1:
                            mm.then_inc(pe, 1)
                tensor.wait_ge(ac, 9)
                for jb in range(NB):
                    for ob in range(NB):
                        mm = tensor.matmul(
                            ps[:, jb, :],
                            lhsT=ee[ob][:, jb * P : (jb + 1) * P],
                            rhs=ed[ob][:, :],
                            start=(ob == 0),
                            stop=(ob == NB - 1),
                        )
                        if ob == NB - 1:
                            mm.then_inc(pe, 1)
                tensor.wait_ge(gA3, 16)
                tensor.wait_ge(dv, NB)
                for ob in range(NB):
                    for jb in range(NB):
                        if ob == 0:
                            tensor.wait_ge(ac, 9 + jb + 1)
                        mm = tensor.matmul(
                            pc(ob),
                            lhsT=eN[:, jb, ob * P : (ob + 1) * P],
                            rhs=lt[jb][:, :],
                            start=(jb == 0),
                            stop=(jb == NB - 1),
                        )
                        if jb == NB - 1:
                            mm.then_inc(pe, 1)

            @block.vector
            def _(vector):
                vector.memset(jbf[:, :], 0.0).then_inc(jz, 1)
                for jb in range(NB):
                    vector.memset(lt[jb][:, T_DEC : T_DEC + 1], 1.0).then_inc(
                        dv, 1
                    )
                for ob in range(NB):
                    vector.wait_ge(ac, 6 + ob)
                    vector.tensor_tensor(
                        out=junk[:, ob, :],
                        in0=pp[ob][:, :],
                        in1=eT(ob)[:, :],
                        op=ALU.mult,
                    ).then_inc(dv, 1)
                    vector.wait_ge(dv, 5 + 2 * ob)
                    vector.reduce_sum(
                        out=cp[:, ob : ob + 1],
                        in_=junk[:, ob, :],
                        axis=mybir.AxisListType.X,
                    ).then_inc(dv, 1)
                vector.wait_ge(gA2, 16)
                for ob in range(NB):
                    vector.wait_ge(pe, 14 if ob < 2 else 16)
                    vector.tensor_copy(
                        se[:, ob : ob + 1], pc(ob)[:, T_DEC : T_DEC + 1]
                    ).then_inc(dv, 1)
                    vector.wait_ge(dv, 13 + 4 * ob)
                    vector.tensor_scalar(
                        out=fx[:, ob : ob + 1],
                        in0=se[:, ob : ob + 1],
                        scalar1=b4[:, ob : ob + 1],
                        scalar2=cp[:, ob : ob + 1],
                        op0=ALU.mult,
                        op1=ALU.add,
                    ).then_inc(dv, 1)
                    vector.wait_ge(dv, 14 + 4 * ob)
                    vector.tensor_scalar(
                        out=ctmp[:, ob, :],
                        in0=pd[:, ob, :],
                        scalar1=se[:, ob : ob + 1],
                        scalar2=fx[:, ob : ob + 1],
                        op0=ALU.mult,
                        op1=ALU.add,
                    ).then_inc(dv, 1)
                    vector.wait_ge(dv, 15 + 4 * ob)
                    vector.tensor_tensor(
                        out=ctxo[:, ob, :],
                        in0=ctmp[:, ob, :],
                        in1=pc(ob)[:, 0:T_DEC],
                        op=ALU.subtract,
                    ).then_inc(dv, 1)

        nc.finalize()
    return nc


def _build_nc():
    return build_raw(bacc, mybir, bass)


def _prep_in_maps(encoderOutput, decoderInput, W, b):
    bf = ml_dtypes.bfloat16
    WT = np.ascontiguousarray(np.asarray(W, np.float32).T)
    b4 = np.ascontiguousarray(np.asarray(b, np.float32).reshape(NB, P).T)
    in_maps = []
    for core in range(B):
        e = np.asarray(encoderOutput[core], np.float32)
        d = np.asarray(decoderInput[core], np.float32)
        eT = e.T
        dT = d.T
        a1 = np.empty((NB, 2, P, H2), np.float32)
        a1[:, 0] = WT[H2:].reshape(NB, P, H2)
        a1[:, 1] = eT.reshape(NB, P, T_ENC)
        bA = np.empty((NB, P, H2 + T_DEC), np.float32)
        bA[:, :, :H2] = WT[:H2].reshape(NB, P, H2)
        bA[:, :, H2:] = dT.reshape(NB, P, T_DEC)
        in_maps.append(
            {
                "encN": e.astype(bf),
                "a1blk": a1.astype(bf),
                "blkA": bA.astype(bf),
                "b4": b4,
            }
        )
    return in_maps


def kernel(encoderOutput, decoderInput, W, b, _trace=False):
    if "nc" not in _CACHE:
        _CACHE["nc"] = _build_nc()
    nc = _CACHE["nc"]
    in_maps = _prep_in_maps(encoderOutput, decoderInput, W, b)
    res = run_bass_kernel_spmd(nc, in_maps, core_ids=list(range(B)), trace=_trace)
    outs = np.stack([np.asarray(r["out"], np.float32).T for r in res.results])
    if _trace:
        _CACHE["last_result"] = res
    return outs


# revision 13
# speedup vs baseline: 1.0133x; 1.0133x over previous
"""Baseline (v1) kernel, reconstructed for device sanity checks."""

import sys

for _p in ("/opt/trn_rl_repo",):
    if _p not in sys.path:
        sys.path.insert(0, _p)

import numpy as np
import ml_dtypes

import concourse.bass as bass
import concourse.tile as tile
from concourse import bacc, mybir
from concourse.bass_utils import run_bass_kernel_spmd
from concourse.hw_specs import get_activation_tables

B, T_DEC, T_ENC, H2 = 8, 64, 512, 512
P = 128
NB = H2 // P

BF16 = mybir.dt.bfloat16
F32 = mybir.dt.float32
AF = mybir.ActivationFunctionType
ALU = mybir.AluOpType

_CACHE = {}

from contextlib import ExitStack

BA_W = H2 + T_DEC


def build_raw(bacc, mybir, bass):
    BF16 = mybir.dt.bfloat16
    F32 = mybir.dt.float32
    AF = mybir.ActivationFunctionType
    ALU = mybir.AluOpType

    nc = bacc.Bacc(None, target_bir_lowering=False)
    tabs = get_activation_tables(nc.m.arch)
    keep = "natural_log_exp_and_others"
    if keep in tabs and AF.Exp in tabs[keep] and AF.Ln in tabs[keep]:
        for name, st in tabs.items():
            if name != keep:
                st.discard(AF.Exp)
                st.discard(AF.Ln)

    encN = nc.dram_tensor("encN", [T_ENC, H2], BF16, kind="ExternalInput")
    a1blk = nc.dram_tensor("a1blk", [NB, 2, P, H2], BF16, kind="ExternalInput")
    blkA = nc.dram_tensor("blkA", [NB, P, BA_W], BF16, kind="ExternalInput")
    b4d = nc.dram_tensor("b4", [P, NB], F32, kind="ExternalInput")
    out = nc.dram_tensor("out", [H2, T_DEC], F32, kind="ExternalOutput")

    encN_r = encN[:, :].rearrange("(a p) o -> p a o", p=P)
    a1_r = a1blk[:, :, :, :].rearrange("a two p c -> p a two c")
    bA_r = blkA[:, :, :].rearrange("a p c -> p a c")
    out_r = out[:, :].rearrange("(a p) i -> p a i", p=P)

    with ExitStack() as ctx:
        ec = ctx.enter_context
        a1t = [ec(nc.sbuf_tensor(f"a1t{d}", [P, 2, H2], BF16)) for d in range(NB)]
        bt = ec(nc.sbuf_tensor("bt", [P, NB, BA_W], BF16))
        eN = ec(nc.sbuf_tensor("eN", [P, NB, H2], BF16))
        b4 = ec(nc.sbuf_tensor("b4s", [P, NB], F32))
        ee = [ec(nc.sbuf_tensor(f"ee{o}", [P, T_ENC], BF16)) for o in range(NB)]
        ed = [ec(nc.sbuf_tensor(f"ed{o}", [P, T_DEC], BF16)) for o in range(NB)]
        lt = [ec(nc.sbuf_tensor(f"lt{j}", [P, T_DEC + 1], BF16)) for j in range(NB)]
        junk = ec(nc.sbuf_tensor("junk", [P, NB, T_ENC], BF16))
        jbf = ec(nc.sbuf_tensor("jbf", [P, T_ENC], BF16))
        wj = ec(nc.sbuf_tensor("wj", [P, NB], F32))
        cp = ec(nc.sbuf_tensor("cp", [P, NB], F32))
        se = ec(nc.sbuf_tensor("se", [P, NB], F32))
        fx = ec(nc.sbuf_tensor("fx", [P, NB], F32))
        ctmp = ec(nc.sbuf_tensor("ctmp", [P, NB, T_DEC], F32))
        ctxo = ec(nc.sbuf_tensor("ctxo", [P, NB, T_DEC], F32))
        pp = [ec(nc.psum_tensor(f"pp{o}", [P, T_ENC], F32)) for o in range(NB)]
        pd = ec(nc.psum_tensor("pd", [P, NB, T_DEC], F32))
        ps = ec(nc.psum_tensor("ps", [P, NB, T_DEC], F32))
        pcA = ec(nc.psum_tensor("pcA", [P, 2, T_DEC + 1], F32))
        pcB = ec(nc.psum_tensor("pcB", [P, 2, T_DEC + 1], F32))

        def pc(ob):
            return (pcA if ob < 2 else pcB)[:, ob % 2, :]

        def wte(db):
            return a1t[db][:, 0, :]

        def eT(db):
            return a1t[db][:, 1, :]

        jz = ec(nc.semaphore("jz"))
        dS = [ec(nc.semaphore(f"dS{d}")) for d in range(NB)]
        gA1 = ec(nc.semaphore("gA1"))
        gA2 = ec(nc.semaphore("gA2"))
        gA3 = ec(nc.semaphore("gA3"))
        dO = ec(nc.semaphore("dO"))
        pe = ec(nc.semaphore("pe"))
        ac = ec(nc.semaphore("ac"))
        dv = ec(nc.semaphore("dv"))

        with nc.Block(no_gpsimd_drain=True) as block:

            @block.sync
            def _(sync):
                for db in range(NB):
                    sync.dma_start(
                        out=a1t[db][:, :, :], in_=a1_r[:, db, :, :]
                    ).then_inc(dS[db], 16)
                sync.wait_ge(dv, 20)
                sync.dma_start(out=out_r[:, 0:2, :], in_=ctxo[:, 0:2, :]).then_inc(
                    dO, 16
                )
                sync.wait_ge(dv, 28)
                sync.dma_start(out=out_r[:, 2:NB, :], in_=ctxo[:, 2:NB, :]).then_inc(
                    dO, 16
                )
                sync.wait_ge(dO, 32)

            @block.gpsimd
            def _(gpsimd):
                gpsimd.dma_start(out=eN[:, :, :], in_=encN_r[:, :, :]).then_inc(
                    gA3, 16
                )

            @block.scalar
            def _(scalar):
                scalar.dma_start(out=bt[:, :, :], in_=bA_r[:, :, :]).then_inc(gA1, 16)
                scalar.dma_start(out=b4[:, :], in_=b4d[:, :]).then_inc(gA2, 16)
                scalar.activation(wj[:, 0:1], wj[:, 3:4], AF.Exp, scale=0.0).then_inc(
                    ac, 1
                )
                scalar.wait_ge(pe, NB)
                for ob in range(NB):
                    scalar.activation(ed[ob][:, :], pd[:, ob, :], AF.Exp).then_inc(
                        ac, 1
                    )
                scalar.wait_ge(gA2, 16)
                for ob in range(NB):
                    scalar.wait_ge(pe, NB + 1 + ob)
                    scalar.activation(
                        ee[ob][:, :],
                        pp[ob][:, :],
                        AF.Exp,
                        bias=b4[:, ob : ob + 1],
                    ).then_inc(ac, 1)
                scalar.activation(wj[:, 1:2], wj[:, 3:4], AF.Ln, bias=1.0, scale=0.0)
                scalar.wait_ge(pe, 3 * NB)
                for jb in range(NB):
                    scalar.activation(lt[jb][:, 0:T_DEC], ps[:, jb, :], AF.Ln).then_inc(
                        ac, 1
                    )

            @block.tensor
            def _(tensor):
                tensor.wait_ge(jz, 1)
                for k in range(10):
                    tensor.matmul(
                        pp[k % NB][:, :],
                        lhsT=jbf[:, 0:P],
                        rhs=jbf[:, :],
                        start=True,
                        stop=True,
                    )
                tensor.wait_ge(gA1, 16)
                for ob in range(NB):
                    for db in range(NB):
                        mm = tensor.matmul(
                            pd[:, ob, :],
                            lhsT=bt[:, db, ob * P : (ob + 1) * P],
                            rhs=bt[:, db, H2 : H2 + T_DEC],
                            start=(db == 0),
                            stop=(db == NB - 1),
                        )
                        if db == NB - 1:
                            mm.then_inc(pe, 1)
                for db in range(NB):
                    tensor.wait_ge(dS[db], 16)
                    for ob in range(NB):
                        mm = tensor.matmul(
                            pp[ob][:, :],
                            lhsT=wte(db)[:, ob * P : (ob + 1) * P],
                            rhs=eT(db)[:, :],
                            start=(db == 0),
                            stop=(db == NB - 1),
                        )
                        if db == NB - 1:
                            mm.then_inc(pe, 1)
                tensor.wait_ge(ac, 9)
                for jb in range(NB):
                    for ob in range(NB):
                        mm = tensor.matmul(
                            ps[:, jb, :],
                            lhsT=ee[ob][:, jb * P : (jb + 1) * P],
                            rhs=ed[ob][:, :],
                            start=(ob == 0),
                            stop=(ob == NB - 1),
                        )
                        if ob == NB - 1:
                            mm.then_inc(pe, 1)
                tensor.wait_ge(gA3, 16)
                tensor.wait_ge(dv, NB)
                for ob in range(NB):
                    for jb in range(NB):
                        if ob == 0:
                            tensor.wait_ge(ac, 9 + jb + 1)
                        mm = tensor.matmul(
                            pc(ob),
                            lhsT=eN[:, jb, ob * P : (ob + 1) * P],
                            rhs=lt[jb][:, :],
                            start=(jb == 0),
                            stop=(jb == NB - 1),
                        )
                        if jb == NB - 1:
                            mm.then_inc(pe, 1)

            @block.vector
            def _(vector):
                vector.memset(jbf[:, :], 0.0).then_inc(jz, 1)
                for jb in range(NB):
                    vector.memset(lt[jb][:, T_DEC : T_DEC + 1], 1.0).then_inc(
                        dv, 1
                    )
                for ob in range(NB):
                    vector.wait_ge(ac, 6 + ob)
                    vector.tensor_tensor(
                        out=junk[:, ob, :],
                        in0=pp[ob][:, :],
                        in1=eT(ob)[:, :],
                        op=ALU.mult,
                    ).then_inc(dv, 1)
                    vector.wait_ge(dv, 5 + 2 * ob)
                    vector.reduce_sum(
                        out=cp[:, ob : ob + 1],
                        in_=junk[:, ob, :],
                        axis=mybir.AxisListType.X,
                    ).then_inc(dv, 1)
                vector.wait_ge(gA2, 16)
                for ob in range(NB):
                    vector.wait_ge(pe, 14 if ob < 2 else 16)
                    vector.tensor_copy(
                        se[:, ob : ob + 1], pc(ob)[:, T_DEC : T_DEC + 1]
                    ).then_inc(dv, 1)
                    vector.wait_ge(dv, 13 + 4 * ob)
                    vector.tensor_scalar(
                        out=fx[:, ob : ob + 1],
                        in0=se[:, ob : ob + 1],
                        scalar1=b4[:, ob : ob + 1],
                        scalar2=cp[:, ob : ob + 1],
                        op0=ALU.mult,
                        op1=ALU.add,
                    ).then_inc(dv, 1)
                    vector.wait_ge(dv, 14 + 4 * ob)
                    vector.tensor_scalar(
                        out=ctmp[:, ob, :],
                        in0=pd[:, ob, :],
                        scalar1=se[:, ob : ob + 1],
                        scalar2=fx[:, ob : ob + 1],
                        op0=ALU.mult,
                        op1=ALU.add,
                    ).then_inc(dv, 1)
                    vector.wait_ge(dv, 15 + 4 * ob)
                    vector.tensor_tensor(
                        out=ctxo[:, ob, :],
                        in0=ctmp[:, ob, :],
                        in1=pc(ob)[:, 0:T_DEC],
                        op=ALU.subtract,
                    ).then_inc(dv, 1)

        nc.finalize()
    return nc


def _build_nc():
    return build_raw(bacc, mybir, bass)


def _prep_in_maps(encoderOutput, decoderInput, W, b):
    bf = ml_dtypes.bfloat16
    WT = np.ascontiguousarray(np.asarray(W, np.float32).T)
    b4 = np.ascontiguousarray(np.asarray(b, np.float32).reshape(NB, P).T)
    in_maps = []
    for core in range(B):
        e = np.asarray(encoderOutput[core], np.float32)
        d = np.asarray(decoderInput[core], np.float32)
        eT = e.T
        dT = d.T
        a1 = np.empty((NB, 2, P, H2), np.float32)
        a1[:, 0] = WT[H2:].reshape(NB, P, H2)
        a1[:, 1] = eT.reshape(NB, P, T_ENC)
        bA = np.empty((NB, P, H2 + T_DEC), np.float32)
        bA[:, :, :H2] = WT[:H2].reshape(NB, P, H2)
        bA[:, :, H2:] = dT.reshape(NB, P, T_DEC)
        in_maps.append(
            {
                "encN": e.astype(bf),
                "a1blk": a1.astype(bf),
                "blkA": bA.astype(bf),
                "b4": b4,
            }
        )
    return in_maps


def kernel(encoderOutput, decoderInput, W, b, _trace=False):
    if "nc" not in _CACHE:
        _CACHE["nc"] = _build_nc()
    nc = _CACHE["nc"]
    in_maps = _prep_in_maps(encoderOutput, decoderInput, W, b)
    res = run_bass_kernel_spmd(nc, in_maps, core_ids=list(range(B)), trace=_trace)
    outs = np.stack([np.asarray(r["out"], np.float32).T for r in res.results])
    if _trace:
        _CACHE["last_result"] = res
    return outs
